# revision 17
# baseline (speedup 1.0000x reference)
"""Trainium2 Bass kernel for nn_BasisSlotAffinityGAT.

Math (per reference):
  z = concat(desc, nv) @ W_fusion.T + b_fusion            [B,N,D]
  S = softmax(z @ W_slot.T + b_slot, -1)                  [B,N,K]
  G = zero_diag(sinkhorn(softmax(G_param)))               [H,K,K]  (tiny; host)
  A = einsum('bnk,hkl,bml->bhnm', S, G, S) / TAU          [B,H,N,N]
  Q = softmax(A, -1); bias_log = log(Q)                   [B,H,N,N] each
  total_reg = orth + usage + frob regularizer scalars

Distribution: data-parallel over B across 8 cores (4 batches/core); weights
and G replicated.  The big outputs Q and bias_log (64MB each) are produced on
device; G/sinkhorn and the regularizer scalars (tiny) are computed on host
from the device-produced S.

Device design notes:
  - All matmul operands are float32r (single-pass fp32 on the PE; tf32-class
    precision) -- plain fp32 matmuls lower to 2 passes + ldweights and are 4x
    the instruction count/cycles.
  - Activations are pre-transposed on host to [d, n] so the fusion matmul
    contracts over partitions without on-device transposes.
  - Slot logits are computed directly in transposed layout LT = [k, n];
    the partition-dim softmax sum uses an all-ones matmul.
  - S^T is replicated to partition offsets {0,32,64,96} with a replication
    matmul so per-head A matmuls (contraction K=16) use PE row-tiling.
  - A-tiles live in [128, 1024] PSUM slots (4 tiles each, 2 banks).  Row
    softmax skips max-subtraction (A/TAU is in [0,2] by construction):
    exp is one batched ACT op per slot; rowsums are GpSimd tensor_reduce
    over the exp result in SBUF; bias_log = A - log(rowsum) is one
    broadcast tensor_tensor subtract per slot; Q = E * (1/rowsum) is one
    broadcast multiply per batch.
  - Post-compile, all ACT table loads are retargeted to the combined
    natural_log_exp_and_others set and deduplicated (the stock pass
    thrashes between the exp-only and ln-only sets, ~1.3us per reload).
"""

import sys

if "/opt/trn_rl_repo" not in sys.path:
    sys.path.insert(0, "/opt/trn_rl_repo")

import numpy as np

import concourse.bass as bass
import concourse.bacc as bacc
import concourse.tile as tile
from concourse import mybir
from concourse.bass_utils import run_bass_kernel_spmd

B, N, D, H, K = 32, 256, 256, 8, 16
NCORES = 8
BLOC = B // NCORES  # batches per core
EPS = 1e-8
SINK_EPS = 1e-6
SINK_ITERS = 10
TAU_SLOT = 0.5
FROB_LAMBDA = 0.02
ORTH_LAMBDA = 0.1
USAGE_LAMBDA = 0.1

F32 = mybir.dt.float32
F32R = mybir.dt.float32r
AF = mybir.ActivationFunctionType
ALU = mybir.AluOpType

_PROGRAM = None


def _bcast(ap, n):
    """Append a step-0 (broadcast) innermost dim of size n to an AP."""
    return bass.AP(tensor=ap.tensor, offset=ap.offset, ap=list(ap.ap) + [[0, n]])


def _retarget_act_tables(nc):
    """Point every ACT table load at the exp+ln combined set; drop duplicates."""
    from concourse.hw_specs import get_activation_tables

    names = list(get_activation_tables(nc.m.arch).keys())
    target = names.index("natural_log_exp_and_others")
    seen = False
    for blk in nc.main_func.blocks:
        keep = []
        for ins in blk.instructions:
            if isinstance(ins, mybir.InstLoadActFuncSet):
                si = ins.sync_info
                plain = si is None or (not si.on_wait and not si.on_update)
                if seen and plain:
                    continue
                ins.act_func_set_id = target
                seen = True
            keep.append(ins)
        blk.instructions[:] = keep


def _build_program():
    nc = bacc.Bacc("TRN2", target_bir_lowering=False, debug=False, num_devices=NCORES)

    # Per-core inputs.  Activations pre-transposed to [b, dchunk, p, n].
    descT = nc.dram_tensor("descT", [BLOC, 2, 128, N], F32R, kind="ExternalInput")
    nvT = nc.dram_tensor("nvT", [BLOC, 2, 128, N], F32R, kind="ExternalInput")
    # Replicated weights, pre-transposed/packed on host.
    w1t = nc.dram_tensor("w1t", [128, 2, D], F32R, kind="ExternalInput")
    w2t = nc.dram_tensor("w2t", [128, 2, D], F32R, kind="ExternalInput")
    wst = nc.dram_tensor("wst", [128, 2, K], F32R, kind="ExternalInput")
    bfu = nc.dram_tensor("bfu", [128, 2], F32, kind="ExternalInput")
    bsl = nc.dram_tensor("bsl", [K, 1], F32, kind="ExternalInput")
    # gpad[k, 128*g + 32*j + l] = G[4g+j, k, l] / TAU ; rep = 4x stacked I16.
    gpad = nc.dram_tensor("gpad", [K, 256], F32R, kind="ExternalInput")
    rep = nc.dram_tensor("rep", [K, 128], F32R, kind="ExternalInput")
    onesd = nc.dram_tensor("onesd", [K, K], F32R, kind="ExternalInput")
    # Per-core outputs.
    # Contiguous-per-partition layout [b, p, h*2+c, m]; host re-permutes to
    # [b, h, (c p), m] afterwards (host time is cheap, HBM descriptors are not).
    q_out = nc.dram_tensor("q_out", [BLOC, 128, H * 2, N], F32, kind="ExternalOutput")
    bl_out = nc.dram_tensor("bl_out", [BLOC, 128, H * 2, N], F32, kind="ExternalOutput")
    st_out = nc.dram_tensor("st_out", [BLOC, K, N], F32R, kind="ExternalOutput")

    with tile.TileContext(nc) as tc:
        with (
            tc.tile_pool(name="consts", bufs=1) as consts,
            tc.tile_pool(name="xin", bufs=4) as xin,
            tc.tile_pool(name="zt", bufs=2) as ztp,
            tc.tile_pool(name="sm", bufs=3) as smp,
            tc.tile_pool(name="big", bufs=2) as bigp,
            tc.tile_pool(name="pz", bufs=1, space="PSUM") as pz,
            tc.tile_pool(name="pm", bufs=3, space="PSUM") as pm,
            tc.tile_pool(name="pa", bufs=2, space="PSUM") as pa,
        ):
            w1_sb = consts.tile([128, 2, D], F32R)
            nc.sync.dma_start(out=w1_sb[:], in_=w1t[:])
            w2_sb = consts.tile([128, 2, D], F32R)
            nc.sync.dma_start(out=w2_sb[:], in_=w2t[:])
            ws_sb = consts.tile([128, 2, K], F32R)
            nc.sync.dma_start(out=ws_sb[:], in_=wst[:])
            bf_sb = consts.tile([128, 2], F32)
            nc.sync.dma_start(out=bf_sb[:], in_=bfu[:])
            bs_sb = consts.tile([K, 1], F32)
            nc.sync.dma_start(out=bs_sb[:], in_=bsl[:])
            gp_sb = consts.tile([K, 256], F32R)
            nc.sync.dma_start(out=gp_sb[:], in_=gpad[:])
            rep_sb = consts.tile([K, 128], F32R)
            nc.sync.dma_start(out=rep_sb[:], in_=rep[:])
            ones_sb = consts.tile([K, K], F32R)
            nc.sync.dma_start(out=ones_sb[:], in_=onesd[:])

            for b in range(BLOC):
                # ---- load activations (pre-transposed [d, n]) ----
                xd = xin.tile([128, 2, N], F32R, tag="xd")
                nc.sync.dma_start(out=xd[:], in_=descT[b].rearrange("c p n -> p c n"))
                xn = xin.tile([128, 2, N], F32R, tag="xn")
                nc.sync.dma_start(out=xn[:], in_=nvT[b].rearrange("c p n -> p c n"))

                # ---- fused projection: zT[o, n] = W1 @ descT + W2 @ nvT ----
                zt_sb = ztp.tile([128, 2, N], F32R, tag="zt")
                zt_ps = pz.tile([128, 2, N], F32, tag="zt_ps")
                for oc in range(2):
                    for i, (w_sb, x_sb) in enumerate(
                        [(w1_sb, xd), (w1_sb, xd), (w2_sb, xn), (w2_sb, xn)]
                    ):
                        c = i % 2
                        nc.tensor.matmul(
                            zt_ps[:, oc, :],
                            lhsT=w_sb[:, c, oc * 128 : (oc + 1) * 128],
                            rhs=x_sb[:, c, :],
                            start=(i == 0),
                            stop=(i == 3),
                        )
                    # PSUM -> SBUF with fused b_fusion add.
                    nc.scalar.activation(
                        out=zt_sb[:, oc, :],
                        in_=zt_ps[:, oc, :],
                        func=AF.Identity,
                        bias=bf_sb[:, oc : oc + 1],
                        scale=1.0,
                    )

                # ---- slot logits (transposed): LT[k, n] ----
                lt_ps = pm.tile([128, N], F32, tag="pmisc")
                for oc in range(2):
                    nc.tensor.matmul(
                        lt_ps[:K, :],
                        lhsT=ws_sb[:, oc, :],
                        rhs=zt_sb[:, oc, :],
                        start=(oc == 0),
                        stop=(oc == 1),
                    )
                # S^T via column softmax (no max-subtraction; logits are small)
                es_sb = smp.tile([K, N], F32R, tag="es")
                nc.scalar.activation(
                    out=es_sb[:],
                    in_=lt_ps[:K, :],
                    func=AF.Exp,
                    bias=bs_sb[:, 0:1],
                    scale=1.0,
                )
                sum_ps = pm.tile([128, N], F32, tag="pmisc")
                nc.tensor.matmul(
                    sum_ps[:K, :], lhsT=ones_sb[:], rhs=es_sb[:], start=True, stop=True
                )
                rs_sb = smp.tile([K, N], F32, tag="rs")
                nc.vector.reciprocal(out=rs_sb[:], in_=sum_ps[:K, :])
                st_sb = smp.tile([K, N], F32R, tag="st")
                nc.vector.tensor_mul(out=st_sb[:], in0=es_sb[:], in1=rs_sb[:])
                nc.sync.dma_start(out=st_out[b], in_=st_sb[:])

                # ---- replicate S^T to partition offsets 0/32/64/96 ----
                strep_ps = pm.tile([128, N], F32, tag="pmisc")
                nc.tensor.matmul(
                    strep_ps[:], lhsT=rep_sb[:], rhs=st_sb[:], start=True, stop=True
                )
                strep_sb = smp.tile([128, N], F32R, tag="strep")
                nc.scalar.copy(out=strep_sb[:], in_=strep_ps[:])

                # ---- SGT[32*j + l, n] = sum_k G'[j, k, l] S^T[k, n], 4 heads/group ----
                sgt_sb = smp.tile([128, 2, N], F32R, tag="sgt")
                for g in range(2):
                    sgt_ps = pm.tile([128, N], F32, tag="pmisc")
                    nc.tensor.matmul(
                        sgt_ps[:],
                        lhsT=gp_sb[:, g * 128 : (g + 1) * 128],
                        rhs=st_sb[:],
                        start=True,
                        stop=True,
                    )
                    nc.scalar.copy(out=sgt_sb[:, g, :], in_=sgt_ps[:])

                # ---- affinity + row softmax + log ----
                # 16 A-tiles per batch, 4 per PSUM slot (2 heads per slot).
                e_bt = bigp.tile([128, H, 2, N], F32, tag="e")
                q_bt = bigp.tile([128, H, 2, N], F32, tag="q")
                l_bt = bigp.tile([128, H, 2, N], F32, tag="l")
                rsum = smp.tile([128, 16], F32, tag="rsum")
                rlog = smp.tile([128, 16], F32, tag="rlog")
                rlogneg = smp.tile([128, 4], F32, tag="rlogneg")
                rrec = smp.tile([128, 16], F32, tag="rrec")

                for grp in range(4):  # heads (2*grp, 2*grp+1)
                    a_ps = pa.tile([128, 4, N], F32, tag="a_ps")
                    for k in range(4):
                        h = grp * 2 + k // 2
                        c2 = k % 2
                        g, j = divmod(h, 4)
                        nc.tensor.matmul(
                            a_ps[:, k, :],
                            lhsT=sgt_sb[
                                32 * j : 32 * j + K, g, c2 * 128 : (c2 + 1) * 128
                            ],
                            rhs=strep_sb[32 * j : 32 * j + K, :],
                            start=True,
                            stop=True,
                            tile_position=(32 * j, 0),
                        )
                    if grp == 1:
                        # per-tile exp with fused accumulator rowsums (ScalarE)
                        for k in range(4):
                            h, c2 = divmod(grp * 4 + k, 2)
                            nc.scalar.activation(
                                out=e_bt[:, h, c2, :],
                                in_=a_ps[:, k, :],
                                func=AF.Exp,
                                accum_out=rsum[:, grp * 4 + k : grp * 4 + k + 1],
                            )
                    else:
                        # batched exp of the whole slot + DVE rowsums
                        nc.scalar.activation(
                            out=e_bt[:, 2 * grp : 2 * grp + 2, :, :],
                            in_=a_ps[:],
                            func=AF.Exp,
                        )
                        nc.vector.reduce_sum(
                            out=rsum[:, 4 * grp : 4 * grp + 4],
                            in_=e_bt[:, 2 * grp : 2 * grp + 2, :, :],
                            axis=mybir.AxisListType.X,
                        )
                    nc.scalar.activation(
                        out=rlog[:, 4 * grp : 4 * grp + 4],
                        in_=rsum[:, 4 * grp : 4 * grp + 4],
                        func=AF.Ln,
                    )
                    # bias_log = A - log(rowsum).  Slot 0 handled per-tile on
                    # ScalarE (idle capacity + frees the PSUM slot without DVE);
                    # slots 1-3 as one broadcast subtract on DVE each.
                    if grp == 0:
                        nc.vector.tensor_scalar_mul(
                            out=rlogneg[:, 0:4], in0=rlog[:, 0:4], scalar1=-1.0
                        )
                        for k in range(4):
                            h, c2 = divmod(k, 2)
                            nc.scalar.activation(
                                out=l_bt[:, h, c2, :],
                                in_=a_ps[:, k, :],
                                func=AF.Identity,
                                bias=rlogneg[:, k : k + 1],
                                scale=1.0,
                            )
                    else:
                        nc.vector.tensor_sub(
                            out=l_bt[:, 2 * grp : 2 * grp + 2, :, :],
                            in0=a_ps[:],
                            in1=_bcast(rlog[:, 4 * grp : 4 * grp + 4], N),
                        )

                # Q = E / rowsum per half-batch (GpSimd -- otherwise idle and
                # all operands live in SBUF); outputs stream out per half.
                nc.vector.reciprocal(out=rrec[:], in_=rsum[:])
                for hf in range(2):
                    sl = slice(hf * H, (hf + 1) * H)
                    q_eng = nc.vector if (b == BLOC - 1 and hf == 1) else nc.gpsimd
                    q_eng.tensor_mul(
                        out=q_bt[:, 4 * hf : 4 * hf + 4, :, :],
                        in0=e_bt[:, 4 * hf : 4 * hf + 4, :, :],
                        in1=_bcast2(rrec[:, sl], 4, N),
                    )
                    nc.sync.dma_start(
                        out=bl_out[b, :, sl, :], in_=l_bt[:, 4 * hf : 4 * hf + 4, :, :]
                    )
                    nc.sync.dma_start(
                        out=q_out[b, :, sl, :], in_=q_bt[:, 4 * hf : 4 * hf + 4, :, :]
                    )

    nc.compile()
    _retarget_act_tables(nc)
    return nc


def _bcast2(ap, h, n):
    """[128, h*2] AP viewed as [128, h, 2, n] with the last dim broadcast."""
    p, f = ap.ap
    return bass.AP(
        tensor=ap.tensor,
        offset=ap.offset,
        ap=[p, [f[0] * 2, h], [f[0], 2], [0, n]],
    )


def _get_program():
    global _PROGRAM
    if _PROGRAM is None:
        _PROGRAM = _build_program()
    return _PROGRAM


def _softmax_last(x):
    m = x.max(axis=-1, keepdims=True)
    e = np.exp(x - m)
    return e / e.sum(axis=-1, keepdims=True)


def _host_G(G_param):
    """softmax -> sinkhorn -> zero diagonal, all float32 (matches reference)."""
    g = _softmax_last(np.asarray(G_param, np.float32))
    m = np.maximum(g, np.float32(SINK_EPS))
    for _ in range(SINK_ITERS):
        m = m / (m.sum(axis=-1, keepdims=True) + np.float32(SINK_EPS))
        m = m / (m.sum(axis=-2, keepdims=True) + np.float32(SINK_EPS))
    return m * (np.float32(1.0) - np.eye(K, dtype=np.float32))


def _prep_inputs(desc, nv, W_fusion, b_fusion, W_slot, b_slot, G):
    """Build the per-core in_maps (host-side layout prep + sharding)."""
    desc = np.asarray(desc, np.float32)
    nv = np.asarray(nv, np.float32)
    W_fusion = np.asarray(W_fusion, np.float32)
    b_fusion = np.asarray(b_fusion, np.float32)
    W_slot = np.asarray(W_slot, np.float32)
    b_slot = np.asarray(b_slot, np.float32)

    descT = np.ascontiguousarray(desc.transpose(0, 2, 1)).reshape(B, 2, 128, N)
    nvT = np.ascontiguousarray(nv.transpose(0, 2, 1)).reshape(B, 2, 128, N)

    def wprep(w):  # [o, d] -> [p, c, o] with d = c*128 + p
        return np.ascontiguousarray(w.T.reshape(2, 128, D).transpose(1, 0, 2))

    w1t = wprep(W_fusion[:, :D])
    w2t = wprep(W_fusion[:, D:])
    wst = np.ascontiguousarray(W_slot.T.reshape(2, 128, K).transpose(1, 0, 2))
    bfu = np.ascontiguousarray(b_fusion.reshape(2, 128).T)
    bsl = np.ascontiguousarray(b_slot[:, None])

    gpad = np.zeros((K, 256), np.float32)
    for h in range(H):
        g, j = divmod(h, 4)
        gpad[:, 128 * g + 32 * j : 128 * g + 32 * j + K] = G[h] / np.float32(TAU_SLOT)
    rep = np.zeros((K, 128), np.float32)
    for j in range(4):
        rep[np.arange(K), 32 * j + np.arange(K)] = 1.0

    shared = {
        "w1t": w1t, "w2t": w2t, "wst": wst, "bfu": bfu, "bsl": bsl,
        "gpad": gpad, "rep": rep, "onesd": np.ones((K, K), np.float32),
    }
    in_maps = []
    for i in range(NCORES):
        sl = slice(i * BLOC, (i + 1) * BLOC)
        in_maps.append({"descT": descT[sl], "nvT": nvT[sl], **shared})
    return in_maps


def _host_regs(S, G):
    """Regularizer scalars from S [B,N,K] and G [H,K,K] (float32, as reference)."""
    eye = np.eye(K, dtype=np.float32)
    sts = np.einsum("bnk,bnl->bkl", S, S) / np.float32(N)
    offdiag = sts * (np.float32(1.0) - eye)
    reg_orth = np.float32(ORTH_LAMBDA) * np.mean(offdiag**2, dtype=np.float32)

    u = S.mean(axis=1)
    u = u / (u.sum(axis=-1, keepdims=True) + np.float32(EPS))
    uc = np.maximum(u, np.float32(EPS))
    kl = np.sum(uc * (np.log(uc) - np.log(np.float32(1.0 / K))), axis=-1)
    reg_usage = np.float32(USAGE_LAMBDA) * np.mean(kl, dtype=np.float32)

    v = G.reshape(H, -1)
    nrm = np.sqrt((v * v).sum(axis=1, keepdims=True))
    v = v / np.maximum(nrm, np.float32(1e-8))
    gram = v @ v.T
    g_reg = np.float32(FROB_LAMBDA) * (gram.sum() - np.trace(gram)) / (H * (H - 1))
    return np.float32(reg_orth + reg_usage + g_reg)


def run_on_device(in_maps, **kwargs):
    nc = _get_program()
    return run_bass_kernel_spmd(nc, in_maps, core_ids=list(range(NCORES)), **kwargs)


def kernel(
    desc_embeddings,
    name_value_embeddings,
    W_fusion,
    b_fusion,
    W_slot,
    b_slot,
    G_param,
):
    G = _host_G(G_param)
    in_maps = _prep_inputs(
        desc_embeddings, name_value_embeddings, W_fusion, b_fusion, W_slot, b_slot, G
    )
    res = run_on_device(in_maps)

    def unperm(name):
        a = np.concatenate([res.results[i][name] for i in range(NCORES)], axis=0)
        # [b, p, h*2+c, m] -> [b, h, c*128+p, m]
        a = a.reshape(B, 128, H, 2, N).transpose(0, 2, 3, 1, 4)
        return np.ascontiguousarray(a).reshape(B, H, N, N)

    Q = unperm("q_out")
    bias_log = unperm("bl_out")
    st = np.concatenate([res.results[i]["st_out"] for i in range(NCORES)], axis=0)
    S = np.ascontiguousarray(st.transpose(0, 2, 1))  # [B, N, K]

    total_reg = _host_regs(S, G)
    return bias_log, Q, total_reg


# revision 18
# speedup vs baseline: 1.0074x; 1.0074x over previous
"""Trainium2 Bass kernel for nn_BasisSlotAffinityGAT.

Math (per reference):
  z = concat(desc, nv) @ W_fusion.T + b_fusion            [B,N,D]
  S = softmax(z @ W_slot.T + b_slot, -1)                  [B,N,K]
  G = zero_diag(sinkhorn(softmax(G_param)))               [H,K,K]  (tiny; host)
  A = einsum('bnk,hkl,bml->bhnm', S, G, S) / TAU          [B,H,N,N]
  Q = softmax(A, -1); bias_log = log(Q)                   [B,H,N,N] each
  total_reg = orth + usage + frob regularizer scalars

Distribution: data-parallel over B across 8 cores (4 batches/core); weights
and G replicated.  The big outputs Q and bias_log (64MB each) are produced on
device; G/sinkhorn and the regularizer scalars (tiny) are computed on host
from the device-produced S.

Device design notes:
  - All matmul operands are float32r (single-pass fp32 on the PE; tf32-class
    precision) -- plain fp32 matmuls lower to 2 passes + ldweights and are 4x
    the instruction count/cycles.
  - Activations are pre-transposed on host to [d, n] so the fusion matmul
    contracts over partitions without on-device transposes.
  - Slot logits are computed directly in transposed layout LT = [k, n];
    the partition-dim softmax sum uses an all-ones matmul.
  - S^T is replicated to partition offsets {0,32,64,96} with a replication
    matmul so per-head A matmuls (contraction K=16) use PE row-tiling.
  - A-tiles live in [128, 1024] PSUM slots (4 tiles each, 2 banks).  Row
    softmax skips max-subtraction (A/TAU is in [0,2] by construction):
    exp is one batched ACT op per slot; rowsums are GpSimd tensor_reduce
    over the exp result in SBUF; bias_log = A - log(rowsum) is one
    broadcast tensor_tensor subtract per slot; Q = E * (1/rowsum) is one
    broadcast multiply per batch.
  - Post-compile, all ACT table loads are retargeted to the combined
    natural_log_exp_and_others set and deduplicated (the stock pass
    thrashes between the exp-only and ln-only sets, ~1.3us per reload).
"""

import sys

if "/opt/trn_rl_repo" not in sys.path:
    sys.path.insert(0, "/opt/trn_rl_repo")

import numpy as np

import concourse.bass as bass
import concourse.bacc as bacc
import concourse.tile as tile
from concourse import mybir
from concourse.bass_utils import run_bass_kernel_spmd

B, N, D, H, K = 32, 256, 256, 8, 16
NCORES = 8
BLOC = B // NCORES  # batches per core
EPS = 1e-8
SINK_EPS = 1e-6
SINK_ITERS = 10
TAU_SLOT = 0.5
FROB_LAMBDA = 0.02
ORTH_LAMBDA = 0.1
USAGE_LAMBDA = 0.1

F32 = mybir.dt.float32
F32R = mybir.dt.float32r
AF = mybir.ActivationFunctionType
ALU = mybir.AluOpType

_PROGRAM = None


def _bcast(ap, n):
    """Append a step-0 (broadcast) innermost dim of size n to an AP."""
    return bass.AP(tensor=ap.tensor, offset=ap.offset, ap=list(ap.ap) + [[0, n]])


def _retarget_act_tables(nc):
    """Point every ACT table load at the exp+ln combined set; drop duplicates."""
    from concourse.hw_specs import get_activation_tables

    names = list(get_activation_tables(nc.m.arch).keys())
    target = names.index("natural_log_exp_and_others")
    seen = False
    for blk in nc.main_func.blocks:
        keep = []
        for ins in blk.instructions:
            if isinstance(ins, mybir.InstLoadActFuncSet):
                si = ins.sync_info
                plain = si is None or (not si.on_wait and not si.on_update)
                if seen and plain:
                    continue
                ins.act_func_set_id = target
                seen = True
            keep.append(ins)
        blk.instructions[:] = keep


def _build_program():
    nc = bacc.Bacc("TRN2", target_bir_lowering=False, debug=False, num_devices=NCORES)

    # Per-core inputs.  Activations pre-transposed to [b, dchunk, p, n].
    descT = nc.dram_tensor("descT", [BLOC, 2, 128, N], F32R, kind="ExternalInput")
    nvT = nc.dram_tensor("nvT", [BLOC, 2, 128, N], F32R, kind="ExternalInput")
    # Replicated weights, pre-transposed/packed on host.
    w1t = nc.dram_tensor("w1t", [128, 2, D], F32R, kind="ExternalInput")
    w2t = nc.dram_tensor("w2t", [128, 2, D], F32R, kind="ExternalInput")
    wst = nc.dram_tensor("wst", [128, 2, K], F32R, kind="ExternalInput")
    bfu = nc.dram_tensor("bfu", [128, 2], F32, kind="ExternalInput")
    bsl = nc.dram_tensor("bsl", [K, 1], F32, kind="ExternalInput")
    # gpad[k, 128*g + 32*j + l] = G[4g+j, k, l] / TAU ; rep = 4x stacked I16.
    gpad = nc.dram_tensor("gpad", [K, 256], F32R, kind="ExternalInput")
    rep = nc.dram_tensor("rep", [K, 128], F32R, kind="ExternalInput")
    onesd = nc.dram_tensor("onesd", [K, K], F32R, kind="ExternalInput")
    # Per-core outputs.
    # Contiguous-per-partition layout [b, p, h*2+c, m]; host re-permutes to
    # [b, h, (c p), m] afterwards (host time is cheap, HBM descriptors are not).
    q_out = nc.dram_tensor("q_out", [BLOC, 128, H * 2, N], F32, kind="ExternalOutput")
    bl_out = nc.dram_tensor("bl_out", [BLOC, 128, H * 2, N], F32, kind="ExternalOutput")
    st_out = nc.dram_tensor("st_out", [BLOC, K, N], F32R, kind="ExternalOutput")

    with tile.TileContext(nc) as tc:
        with (
            tc.tile_pool(name="consts", bufs=1) as consts,
            tc.tile_pool(name="xin", bufs=4) as xin,
            tc.tile_pool(name="zt", bufs=2) as ztp,
            tc.tile_pool(name="sm", bufs=3) as smp,
            tc.tile_pool(name="big", bufs=2) as bigp,
            tc.tile_pool(name="pm", bufs=2, space="PSUM") as pm,
            tc.tile_pool(name="pa", bufs=3, space="PSUM") as pa,
        ):
            w1_sb = consts.tile([128, 2, D], F32R)
            nc.sync.dma_start(out=w1_sb[:], in_=w1t[:])
            w2_sb = consts.tile([128, 2, D], F32R)
            nc.sync.dma_start(out=w2_sb[:], in_=w2t[:])
            ws_sb = consts.tile([128, 2, K], F32R)
            nc.sync.dma_start(out=ws_sb[:], in_=wst[:])
            bf_sb = consts.tile([128, 2], F32)
            nc.sync.dma_start(out=bf_sb[:], in_=bfu[:])
            bs_sb = consts.tile([K, 1], F32)
            nc.sync.dma_start(out=bs_sb[:], in_=bsl[:])
            gp_sb = consts.tile([K, 256], F32R)
            nc.sync.dma_start(out=gp_sb[:], in_=gpad[:])
            rep_sb = consts.tile([K, 128], F32R)
            nc.sync.dma_start(out=rep_sb[:], in_=rep[:])
            ones_sb = consts.tile([K, K], F32R)
            nc.sync.dma_start(out=ones_sb[:], in_=onesd[:])

            for b in range(BLOC):
                # ---- load activations (pre-transposed [d, n]) ----
                xd = xin.tile([128, 2, N], F32R, tag="xd")
                nc.sync.dma_start(out=xd[:], in_=descT[b].rearrange("c p n -> p c n"))
                xn = xin.tile([128, 2, N], F32R, tag="xn")
                nc.sync.dma_start(out=xn[:], in_=nvT[b].rearrange("c p n -> p c n"))

                # ---- fused projection: zT[o, n] = W1 @ descT + W2 @ nvT ----
                zt_sb = ztp.tile([128, 2, N], F32R, tag="zt")
                zt_ps = pm.tile([128, 2, N], F32, tag="pmisc")
                for oc in range(2):
                    for i, (w_sb, x_sb) in enumerate(
                        [(w1_sb, xd), (w1_sb, xd), (w2_sb, xn), (w2_sb, xn)]
                    ):
                        c = i % 2
                        nc.tensor.matmul(
                            zt_ps[:, oc, :],
                            lhsT=w_sb[:, c, oc * 128 : (oc + 1) * 128],
                            rhs=x_sb[:, c, :],
                            start=(i == 0),
                            stop=(i == 3),
                        )
                    # PSUM -> SBUF with fused b_fusion add.
                    nc.scalar.activation(
                        out=zt_sb[:, oc, :],
                        in_=zt_ps[:, oc, :],
                        func=AF.Identity,
                        bias=bf_sb[:, oc : oc + 1],
                        scale=1.0,
                    )

                # ---- slot logits (transposed): LT[k, n] ----
                lt_ps = pm.tile([128, N], F32, tag="pmisc")
                for oc in range(2):
                    nc.tensor.matmul(
                        lt_ps[:K, :],
                        lhsT=ws_sb[:, oc, :],
                        rhs=zt_sb[:, oc, :],
                        start=(oc == 0),
                        stop=(oc == 1),
                    )
                # S^T via column softmax (no max-subtraction; logits are small)
                es_sb = smp.tile([K, N], F32R, tag="es")
                nc.scalar.activation(
                    out=es_sb[:],
                    in_=lt_ps[:K, :],
                    func=AF.Exp,
                    bias=bs_sb[:, 0:1],
                    scale=1.0,
                )
                sum_ps = pm.tile([128, N], F32, tag="pmisc")
                nc.tensor.matmul(
                    sum_ps[:K, :], lhsT=ones_sb[:], rhs=es_sb[:], start=True, stop=True
                )
                rs_sb = smp.tile([K, N], F32, tag="rs")
                nc.vector.reciprocal(out=rs_sb[:], in_=sum_ps[:K, :])
                st_sb = smp.tile([K, N], F32R, tag="st")
                nc.vector.tensor_mul(out=st_sb[:], in0=es_sb[:], in1=rs_sb[:])
                nc.sync.dma_start(out=st_out[b], in_=st_sb[:])

                # ---- replicate S^T to partition offsets 0/32/64/96 ----
                strep_ps = pm.tile([128, N], F32, tag="pmisc")
                nc.tensor.matmul(
                    strep_ps[:], lhsT=rep_sb[:], rhs=st_sb[:], start=True, stop=True
                )
                strep_sb = smp.tile([128, N], F32R, tag="strep")
                nc.scalar.copy(out=strep_sb[:], in_=strep_ps[:])

                # ---- SGT[32*j + l, n] = sum_k G'[j, k, l] S^T[k, n], 4 heads/group ----
                sgt_sb = smp.tile([128, 2, N], F32R, tag="sgt")
                for g in range(2):
                    sgt_ps = pm.tile([128, N], F32, tag="pmisc")
                    nc.tensor.matmul(
                        sgt_ps[:],
                        lhsT=gp_sb[:, g * 128 : (g + 1) * 128],
                        rhs=st_sb[:],
                        start=True,
                        stop=True,
                    )
                    nc.scalar.copy(out=sgt_sb[:, g, :], in_=sgt_ps[:])

                # ---- affinity + row softmax + log ----
                # 16 A-tiles per batch, 4 per PSUM slot (2 heads per slot).
                e_bt = bigp.tile([128, H, 2, N], F32, tag="e")
                q_bt = bigp.tile([128, H, 2, N], F32, tag="q")
                l_bt = bigp.tile([128, H, 2, N], F32, tag="l")
                rsum = smp.tile([128, 16], F32, tag="rsum")
                rlog = smp.tile([128, 16], F32, tag="rlog")
                rlogneg = smp.tile([128, 4], F32, tag="rlogneg")
                rrec = smp.tile([128, 16], F32, tag="rrec")

                for grp in range(4):  # heads (2*grp, 2*grp+1)
                    a_ps = pa.tile([128, 4, N], F32, tag="a_ps")
                    for k in range(4):
                        h = grp * 2 + k // 2
                        c2 = k % 2
                        g, j = divmod(h, 4)
                        nc.tensor.matmul(
                            a_ps[:, k, :],
                            lhsT=sgt_sb[
                                32 * j : 32 * j + K, g, c2 * 128 : (c2 + 1) * 128
                            ],
                            rhs=strep_sb[32 * j : 32 * j + K, :],
                            start=True,
                            stop=True,
                            tile_position=(32 * j, 0),
                        )
                    if grp == 1:
                        # per-tile exp with fused accumulator rowsums (ScalarE)
                        for k in range(4):
                            h, c2 = divmod(grp * 4 + k, 2)
                            nc.scalar.activation(
                                out=e_bt[:, h, c2, :],
                                in_=a_ps[:, k, :],
                                func=AF.Exp,
                                accum_out=rsum[:, grp * 4 + k : grp * 4 + k + 1],
                            )
                    else:
                        # batched exp of the whole slot + DVE rowsums
                        nc.scalar.activation(
                            out=e_bt[:, 2 * grp : 2 * grp + 2, :, :],
                            in_=a_ps[:],
                            func=AF.Exp,
                        )
                        nc.vector.reduce_sum(
                            out=rsum[:, 4 * grp : 4 * grp + 4],
                            in_=e_bt[:, 2 * grp : 2 * grp + 2, :, :],
                            axis=mybir.AxisListType.X,
                        )
                    nc.scalar.activation(
                        out=rlog[:, 4 * grp : 4 * grp + 4],
                        in_=rsum[:, 4 * grp : 4 * grp + 4],
                        func=AF.Ln,
                    )
                    # bias_log = A - log(rowsum).  Slot 0 handled per-tile on
                    # ScalarE (idle capacity + frees the PSUM slot without DVE);
                    # slots 1-3 as one broadcast subtract on DVE each.
                    if grp == 0:
                        nc.vector.tensor_scalar_mul(
                            out=rlogneg[:, 0:4], in0=rlog[:, 0:4], scalar1=-1.0
                        )
                        for k in range(4):
                            h, c2 = divmod(k, 2)
                            nc.scalar.activation(
                                out=l_bt[:, h, c2, :],
                                in_=a_ps[:, k, :],
                                func=AF.Identity,
                                bias=rlogneg[:, k : k + 1],
                                scale=1.0,
                            )
                    else:
                        nc.vector.tensor_sub(
                            out=l_bt[:, 2 * grp : 2 * grp + 2, :, :],
                            in0=a_ps[:],
                            in1=_bcast(rlog[:, 4 * grp : 4 * grp + 4], N),
                        )

                # Q = E / rowsum per half-batch (GpSimd -- otherwise idle and
                # all operands live in SBUF); outputs stream out per half.
                nc.vector.reciprocal(out=rrec[:], in_=rsum[:])
                for hf in range(2):
                    sl = slice(hf * H, (hf + 1) * H)
                    q_eng = nc.vector if (b == BLOC - 1 and hf == 1) else nc.gpsimd
                    q_eng.tensor_mul(
                        out=q_bt[:, 4 * hf : 4 * hf + 4, :, :],
                        in0=e_bt[:, 4 * hf : 4 * hf + 4, :, :],
                        in1=_bcast2(rrec[:, sl], 4, N),
                    )
                    nc.sync.dma_start(
                        out=bl_out[b, :, sl, :], in_=l_bt[:, 4 * hf : 4 * hf + 4, :, :]
                    )
                    nc.sync.dma_start(
                        out=q_out[b, :, sl, :], in_=q_bt[:, 4 * hf : 4 * hf + 4, :, :]
                    )

    nc.compile()
    _retarget_act_tables(nc)
    return nc


def _bcast2(ap, h, n):
    """[128, h*2] AP viewed as [128, h, 2, n] with the last dim broadcast."""
    p, f = ap.ap
    return bass.AP(
        tensor=ap.tensor,
        offset=ap.offset,
        ap=[p, [f[0] * 2, h], [f[0], 2], [0, n]],
    )


def _get_program():
    global _PROGRAM
    if _PROGRAM is None:
        _PROGRAM = _build_program()
    return _PROGRAM


def _softmax_last(x):
    m = x.max(axis=-1, keepdims=True)
    e = np.exp(x - m)
    return e / e.sum(axis=-1, keepdims=True)


def _host_G(G_param):
    """softmax -> sinkhorn -> zero diagonal, all float32 (matches reference)."""
    g = _softmax_last(np.asarray(G_param, np.float32))
    m = np.maximum(g, np.float32(SINK_EPS))
    for _ in range(SINK_ITERS):
        m = m / (m.sum(axis=-1, keepdims=True) + np.float32(SINK_EPS))
        m = m / (m.sum(axis=-2, keepdims=True) + np.float32(SINK_EPS))
    return m * (np.float32(1.0) - np.eye(K, dtype=np.float32))


def _prep_inputs(desc, nv, W_fusion, b_fusion, W_slot, b_slot, G):
    """Build the per-core in_maps (host-side layout prep + sharding)."""
    desc = np.asarray(desc, np.float32)
    nv = np.asarray(nv, np.float32)
    W_fusion = np.asarray(W_fusion, np.float32)
    b_fusion = np.asarray(b_fusion, np.float32)
    W_slot = np.asarray(W_slot, np.float32)
    b_slot = np.asarray(b_slot, np.float32)

    descT = np.ascontiguousarray(desc.transpose(0, 2, 1)).reshape(B, 2, 128, N)
    nvT = np.ascontiguousarray(nv.transpose(0, 2, 1)).reshape(B, 2, 128, N)

    def wprep(w):  # [o, d] -> [p, c, o] with d = c*128 + p
        return np.ascontiguousarray(w.T.reshape(2, 128, D).transpose(1, 0, 2))

    w1t = wprep(W_fusion[:, :D])
    w2t = wprep(W_fusion[:, D:])
    wst = np.ascontiguousarray(W_slot.T.reshape(2, 128, K).transpose(1, 0, 2))
    bfu = np.ascontiguousarray(b_fusion.reshape(2, 128).T)
    bsl = np.ascontiguousarray(b_slot[:, None])

    gpad = np.zeros((K, 256), np.float32)
    for h in range(H):
        g, j = divmod(h, 4)
        gpad[:, 128 * g + 32 * j : 128 * g + 32 * j + K] = G[h] / np.float32(TAU_SLOT)
    rep = np.zeros((K, 128), np.float32)
    for j in range(4):
        rep[np.arange(K), 32 * j + np.arange(K)] = 1.0

    shared = {
        "w1t": w1t, "w2t": w2t, "wst": wst, "bfu": bfu, "bsl": bsl,
        "gpad": gpad, "rep": rep, "onesd": np.ones((K, K), np.float32),
    }
    in_maps = []
    for i in range(NCORES):
        sl = slice(i * BLOC, (i + 1) * BLOC)
        in_maps.append({"descT": descT[sl], "nvT": nvT[sl], **shared})
    return in_maps


def _host_regs(S, G):
    """Regularizer scalars from S [B,N,K] and G [H,K,K] (float32, as reference)."""
    eye = np.eye(K, dtype=np.float32)
    sts = np.einsum("bnk,bnl->bkl", S, S) / np.float32(N)
    offdiag = sts * (np.float32(1.0) - eye)
    reg_orth = np.float32(ORTH_LAMBDA) * np.mean(offdiag**2, dtype=np.float32)

    u = S.mean(axis=1)
    u = u / (u.sum(axis=-1, keepdims=True) + np.float32(EPS))
    uc = np.maximum(u, np.float32(EPS))
    kl = np.sum(uc * (np.log(uc) - np.log(np.float32(1.0 / K))), axis=-1)
    reg_usage = np.float32(USAGE_LAMBDA) * np.mean(kl, dtype=np.float32)

    v = G.reshape(H, -1)
    nrm = np.sqrt((v * v).sum(axis=1, keepdims=True))
    v = v / np.maximum(nrm, np.float32(1e-8))
    gram = v @ v.T
    g_reg = np.float32(FROB_LAMBDA) * (gram.sum() - np.trace(gram)) / (H * (H - 1))
    return np.float32(reg_orth + reg_usage + g_reg)


def run_on_device(in_maps, **kwargs):
    nc = _get_program()
    return run_bass_kernel_spmd(nc, in_maps, core_ids=list(range(NCORES)), **kwargs)


def kernel(
    desc_embeddings,
    name_value_embeddings,
    W_fusion,
    b_fusion,
    W_slot,
    b_slot,
    G_param,
):
    G = _host_G(G_param)
    in_maps = _prep_inputs(
        desc_embeddings, name_value_embeddings, W_fusion, b_fusion, W_slot, b_slot, G
    )
    res = run_on_device(in_maps)

    def unperm(name):
        a = np.concatenate([res.results[i][name] for i in range(NCORES)], axis=0)
        # [b, p, h*2+c, m] -> [b, h, c*128+p, m]
        a = a.reshape(B, 128, H, 2, N).transpose(0, 2, 3, 1, 4)
        return np.ascontiguousarray(a).reshape(B, H, N, N)

    Q = unperm("q_out")
    bias_log = unperm("bl_out")
    st = np.concatenate([res.results[i]["st_out"] for i in range(NCORES)], axis=0)
    S = np.ascontiguousarray(st.transpose(0, 2, 1))  # [B, N, K]

    total_reg = _host_regs(S, G)
    return bias_log, Q, total_reg


# revision 19
# speedup vs baseline: 1.0850x; 1.0770x over previous
"""Trainium2 Bass kernel for nn_BasisSlotAffinityGAT.

Math (per reference):
  z = concat(desc, nv) @ W_fusion.T + b_fusion            [B,N,D]
  S = softmax(z @ W_slot.T + b_slot, -1)                  [B,N,K]
  G = zero_diag(sinkhorn(softmax(G_param)))               [H,K,K]  (tiny; host)
  A = einsum('bnk,hkl,bml->bhnm', S, G, S) / TAU          [B,H,N,N]
  Q = softmax(A, -1); bias_log = log(Q)                   [B,H,N,N] each
  total_reg = orth + usage + frob regularizer scalars

Distribution: data-parallel over B across 8 cores (4 batches/core); weights
and G replicated.  The big outputs Q and bias_log (64MB each) are produced on
device; G/sinkhorn and the regularizer scalars (tiny) are computed on host
from the device-produced S.

Device design notes:
  - All matmul operands are float32r (single-pass fp32 on the PE; tf32-class
    precision) -- plain fp32 matmuls lower to 2 passes + ldweights and are 4x
    the instruction count/cycles.
  - Activations are pre-transposed on host to [d, n] so the fusion matmul
    contracts over partitions without on-device transposes.
  - Slot logits are computed directly in transposed layout LT = [k, n];
    the partition-dim softmax sum uses an all-ones matmul.
  - S^T is replicated to partition offsets {0,32,64,96} with a replication
    matmul so per-head A matmuls (contraction K=16) use PE row-tiling.
  - A-tiles live in [128, 1024] PSUM slots (4 tiles each, 2 banks).  Row
    softmax skips max-subtraction (A/TAU is in [0,2] by construction):
    exp is one batched ACT op per slot; rowsums are GpSimd tensor_reduce
    over the exp result in SBUF; bias_log = A - log(rowsum) is one
    broadcast tensor_tensor subtract per slot; Q = E * (1/rowsum) is one
    broadcast multiply per batch.
  - Post-compile, all ACT table loads are retargeted to the combined
    natural_log_exp_and_others set and deduplicated (the stock pass
    thrashes between the exp-only and ln-only sets, ~1.3us per reload).
"""

import sys

if "/opt/trn_rl_repo" not in sys.path:
    sys.path.insert(0, "/opt/trn_rl_repo")

import numpy as np

import concourse.bass as bass
import concourse.bacc as bacc
import concourse.tile as tile
from concourse import mybir
from concourse.bass_utils import run_bass_kernel_spmd

B, N, D, H, K = 32, 256, 256, 8, 16
NCORES = 8
BLOC = B // NCORES  # batches per core
EPS = 1e-8
SINK_EPS = 1e-6
SINK_ITERS = 10
TAU_SLOT = 0.5
FROB_LAMBDA = 0.02
ORTH_LAMBDA = 0.1
USAGE_LAMBDA = 0.1

F32 = mybir.dt.float32
F32R = mybir.dt.float32r
AF = mybir.ActivationFunctionType
ALU = mybir.AluOpType

_PROGRAM = None


def _bcast(ap, n):
    """Append a step-0 (broadcast) innermost dim of size n to an AP."""
    return bass.AP(tensor=ap.tensor, offset=ap.offset, ap=list(ap.ap) + [[0, n]])


def _retarget_act_tables(nc):
    """Point every ACT table load at the exp+ln combined set; drop duplicates."""
    from concourse.hw_specs import get_activation_tables

    names = list(get_activation_tables(nc.m.arch).keys())
    target = names.index("natural_log_exp_and_others")
    seen = False
    for blk in nc.main_func.blocks:
        keep = []
        for ins in blk.instructions:
            if isinstance(ins, mybir.InstLoadActFuncSet):
                si = ins.sync_info
                plain = si is None or (not si.on_wait and not si.on_update)
                if seen and plain:
                    continue
                ins.act_func_set_id = target
                seen = True
            keep.append(ins)
        blk.instructions[:] = keep


def _build_program():
    nc = bacc.Bacc("TRN2", target_bir_lowering=False, debug=False, num_devices=NCORES)

    # Per-core inputs.  Activations pre-transposed to [b, dchunk, p, n].
    descT = nc.dram_tensor("descT", [BLOC, 2, 128, N], F32R, kind="ExternalInput")
    nvT = nc.dram_tensor("nvT", [BLOC, 2, 128, N], F32R, kind="ExternalInput")
    # Replicated weights, pre-transposed/packed on host.
    w1t = nc.dram_tensor("w1t", [128, 2, D], F32R, kind="ExternalInput")
    w2t = nc.dram_tensor("w2t", [128, 2, D], F32R, kind="ExternalInput")
    wst = nc.dram_tensor("wst", [128, 2, K], F32R, kind="ExternalInput")
    bfu = nc.dram_tensor("bfu", [128, 2], F32, kind="ExternalInput")
    bsl = nc.dram_tensor("bsl", [K, 1], F32, kind="ExternalInput")
    # gpad[k, 128*g + 32*j + l] = G[4g+j, k, l] / TAU ; rep = 4x stacked I16.
    gpad = nc.dram_tensor("gpad", [K, 256], F32R, kind="ExternalInput")
    rep = nc.dram_tensor("rep", [K, 128], F32R, kind="ExternalInput")
    onesd = nc.dram_tensor("onesd", [K, K], F32R, kind="ExternalInput")
    # Per-core outputs.
    # Contiguous-per-partition layout [b, p, h*2+c, m]; host re-permutes to
    # [b, h, (c p), m] afterwards (host time is cheap, HBM descriptors are not).
    q_out = nc.dram_tensor("q_out", [BLOC, 128, H * 2, N], F32, kind="ExternalOutput")
    bl_out = nc.dram_tensor("bl_out", [BLOC, 128, H * 2, N], F32, kind="ExternalOutput")
    st_out = nc.dram_tensor("st_out", [BLOC, K, N], F32R, kind="ExternalOutput")

    with tile.TileContext(nc) as tc:
        with (
            tc.tile_pool(name="consts", bufs=1) as consts,
            tc.tile_pool(name="xin", bufs=4) as xin,
            tc.tile_pool(name="zt", bufs=2) as ztp,
            tc.tile_pool(name="sm", bufs=3) as smp,
            tc.tile_pool(name="big", bufs=2) as bigp,
            tc.tile_pool(name="pz", bufs=1, space="PSUM") as pz,
            tc.tile_pool(name="pm", bufs=1, space="PSUM") as pm,
            tc.tile_pool(name="pa", bufs=3, space="PSUM") as pa,
        ):
            w1_sb = consts.tile([128, 2, D], F32R)
            nc.sync.dma_start(out=w1_sb[:], in_=w1t[:])
            w2_sb = consts.tile([128, 2, D], F32R)
            nc.sync.dma_start(out=w2_sb[:], in_=w2t[:])
            ws_sb = consts.tile([128, 2, K], F32R)
            nc.sync.dma_start(out=ws_sb[:], in_=wst[:])
            bf_sb = consts.tile([128, 2], F32)
            nc.sync.dma_start(out=bf_sb[:], in_=bfu[:])
            bs_sb = consts.tile([K, 1], F32)
            nc.sync.dma_start(out=bs_sb[:], in_=bsl[:])
            gp_sb = consts.tile([K, 256], F32R)
            nc.sync.dma_start(out=gp_sb[:], in_=gpad[:])
            rep_sb = consts.tile([K, 128], F32R)
            nc.sync.dma_start(out=rep_sb[:], in_=rep[:])
            ones_sb = consts.tile([K, K], F32R)
            nc.sync.dma_start(out=ones_sb[:], in_=onesd[:])

            for b in range(BLOC):
                # ---- load activations (pre-transposed [d, n]) ----
                xd = xin.tile([128, 2, N], F32R, tag="xd")
                nc.sync.dma_start(out=xd[:], in_=descT[b].rearrange("c p n -> p c n"))
                xn = xin.tile([128, 2, N], F32R, tag="xn")
                nc.sync.dma_start(out=xn[:], in_=nvT[b].rearrange("c p n -> p c n"))

                # ---- fused projection: zT[o, n] = W1 @ descT + W2 @ nvT ----
                zt_sb = ztp.tile([128, 2, N], F32R, tag="zt")
                zt_ps = pz.tile([128, 2, N], F32, tag="zt_ps")
                for oc in range(2):
                    for i, (w_sb, x_sb) in enumerate(
                        [(w1_sb, xd), (w1_sb, xd), (w2_sb, xn), (w2_sb, xn)]
                    ):
                        c = i % 2
                        nc.tensor.matmul(
                            zt_ps[:, oc, :],
                            lhsT=w_sb[:, c, oc * 128 : (oc + 1) * 128],
                            rhs=x_sb[:, c, :],
                            start=(i == 0),
                            stop=(i == 3),
                        )
                    # PSUM -> SBUF with fused b_fusion add.
                    nc.scalar.activation(
                        out=zt_sb[:, oc, :],
                        in_=zt_ps[:, oc, :],
                        func=AF.Identity,
                        bias=bf_sb[:, oc : oc + 1],
                        scale=1.0,
                    )

                # ---- slot logits (transposed): LT[k, n] ----
                lt_ps = pm.tile([128, N], F32, tag="pmisc")
                for oc in range(2):
                    nc.tensor.matmul(
                        lt_ps[:K, :],
                        lhsT=ws_sb[:, oc, :],
                        rhs=zt_sb[:, oc, :],
                        start=(oc == 0),
                        stop=(oc == 1),
                    )
                # S^T via column softmax (no max-subtraction; logits are small)
                es_sb = smp.tile([K, N], F32R, tag="es")
                nc.scalar.activation(
                    out=es_sb[:],
                    in_=lt_ps[:K, :],
                    func=AF.Exp,
                    bias=bs_sb[:, 0:1],
                    scale=1.0,
                )
                sum_ps = pm.tile([128, N], F32, tag="pmisc")
                nc.tensor.matmul(
                    sum_ps[:K, :], lhsT=ones_sb[:], rhs=es_sb[:], start=True, stop=True
                )
                rs_sb = smp.tile([K, N], F32, tag="rs")
                nc.vector.reciprocal(out=rs_sb[:], in_=sum_ps[:K, :])
                st_sb = smp.tile([K, N], F32R, tag="st")
                nc.vector.tensor_mul(out=st_sb[:], in0=es_sb[:], in1=rs_sb[:])
                nc.sync.dma_start(out=st_out[b], in_=st_sb[:])

                # ---- replicate S^T to partition offsets 0/32/64/96 ----
                strep_ps = pm.tile([128, N], F32, tag="pmisc")
                nc.tensor.matmul(
                    strep_ps[:], lhsT=rep_sb[:], rhs=st_sb[:], start=True, stop=True
                )
                strep_sb = smp.tile([128, N], F32R, tag="strep")
                nc.scalar.copy(out=strep_sb[:], in_=strep_ps[:])

                # ---- SGT[32*j + l, n] = sum_k G'[j, k, l] S^T[k, n], 4 heads/group ----
                sgt_sb = smp.tile([128, 2, N], F32R, tag="sgt")
                for g in range(2):
                    sgt_ps = pm.tile([128, N], F32, tag="pmisc")
                    nc.tensor.matmul(
                        sgt_ps[:],
                        lhsT=gp_sb[:, g * 128 : (g + 1) * 128],
                        rhs=st_sb[:],
                        start=True,
                        stop=True,
                    )
                    nc.scalar.copy(out=sgt_sb[:, g, :], in_=sgt_ps[:])

                # ---- affinity + row softmax + log ----
                # 16 A-tiles per batch, 4 per PSUM slot (2 heads per slot).
                e_bt = bigp.tile([128, H, 2, N], F32, tag="e")
                q_bt = bigp.tile([128, H, 2, N], F32, tag="q")
                l_bt = bigp.tile([128, H, 2, N], F32, tag="l")
                rsum = smp.tile([128, 16], F32, tag="rsum")
                rlog = smp.tile([128, 16], F32, tag="rlog")
                rlogneg = smp.tile([128, 4], F32, tag="rlogneg")
                rrec = smp.tile([128, 16], F32, tag="rrec")

                for grp in range(4):  # heads (2*grp, 2*grp+1)
                    a_ps = pa.tile([128, 4, N], F32, tag="a_ps")
                    for k in range(4):
                        h = grp * 2 + k // 2
                        c2 = k % 2
                        g, j = divmod(h, 4)
                        nc.tensor.matmul(
                            a_ps[:, k, :],
                            lhsT=sgt_sb[
                                32 * j : 32 * j + K, g, c2 * 128 : (c2 + 1) * 128
                            ],
                            rhs=strep_sb[32 * j : 32 * j + K, :],
                            start=True,
                            stop=True,
                            tile_position=(32 * j, 0),
                        )
                    if grp == 1:
                        # per-tile exp with fused accumulator rowsums (ScalarE)
                        for k in range(4):
                            h, c2 = divmod(grp * 4 + k, 2)
                            nc.scalar.activation(
                                out=e_bt[:, h, c2, :],
                                in_=a_ps[:, k, :],
                                func=AF.Exp,
                                accum_out=rsum[:, grp * 4 + k : grp * 4 + k + 1],
                            )
                    else:
                        # batched exp of the whole slot + DVE rowsums
                        nc.scalar.activation(
                            out=e_bt[:, 2 * grp : 2 * grp + 2, :, :],
                            in_=a_ps[:],
                            func=AF.Exp,
                        )
                        nc.vector.reduce_sum(
                            out=rsum[:, 4 * grp : 4 * grp + 4],
                            in_=e_bt[:, 2 * grp : 2 * grp + 2, :, :],
                            axis=mybir.AxisListType.X,
                        )
                    nc.scalar.activation(
                        out=rlog[:, 4 * grp : 4 * grp + 4],
                        in_=rsum[:, 4 * grp : 4 * grp + 4],
                        func=AF.Ln,
                    )
                    # bias_log = A - log(rowsum).  Slot 0 handled per-tile on
                    # ScalarE (idle capacity + frees the PSUM slot without DVE);
                    # slots 1-3 as one broadcast subtract on DVE each.
                    if grp == 0:
                        nc.vector.tensor_scalar_mul(
                            out=rlogneg[:, 0:4], in0=rlog[:, 0:4], scalar1=-1.0
                        )
                        for k in range(4):
                            h, c2 = divmod(k, 2)
                            nc.scalar.activation(
                                out=l_bt[:, h, c2, :],
                                in_=a_ps[:, k, :],
                                func=AF.Identity,
                                bias=rlogneg[:, k : k + 1],
                                scale=1.0,
                            )
                    else:
                        nc.vector.tensor_sub(
                            out=l_bt[:, 2 * grp : 2 * grp + 2, :, :],
                            in0=a_ps[:],
                            in1=_bcast(rlog[:, 4 * grp : 4 * grp + 4], N),
                        )

                # Q = E / rowsum per half-batch (GpSimd -- otherwise idle and
                # all operands live in SBUF); outputs stream out per half.
                nc.vector.reciprocal(out=rrec[:], in_=rsum[:])
                for hf in range(2):
                    sl = slice(hf * H, (hf + 1) * H)
                    q_eng = nc.vector if (b == BLOC - 1 and hf == 1) else nc.gpsimd
                    q_eng.tensor_mul(
                        out=q_bt[:, 4 * hf : 4 * hf + 4, :, :],
                        in0=e_bt[:, 4 * hf : 4 * hf + 4, :, :],
                        in1=_bcast2(rrec[:, sl], 4, N),
                    )
                    nc.sync.dma_start(
                        out=bl_out[b, :, sl, :], in_=l_bt[:, 4 * hf : 4 * hf + 4, :, :]
                    )
                    nc.sync.dma_start(
                        out=q_out[b, :, sl, :], in_=q_bt[:, 4 * hf : 4 * hf + 4, :, :]
                    )

    nc.compile()
    _retarget_act_tables(nc)
    return nc


def _bcast2(ap, h, n):
    """[128, h*2] AP viewed as [128, h, 2, n] with the last dim broadcast."""
    p, f = ap.ap
    return bass.AP(
        tensor=ap.tensor,
        offset=ap.offset,
        ap=[p, [f[0] * 2, h], [f[0], 2], [0, n]],
    )


def _get_program():
    global _PROGRAM
    if _PROGRAM is None:
        _PROGRAM = _build_program()
    return _PROGRAM


def _softmax_last(x):
    m = x.max(axis=-1, keepdims=True)
    e = np.exp(x - m)
    return e / e.sum(axis=-1, keepdims=True)


def _host_G(G_param):
    """softmax -> sinkhorn -> zero diagonal, all float32 (matches reference)."""
    g = _softmax_last(np.asarray(G_param, np.float32))
    m = np.maximum(g, np.float32(SINK_EPS))
    for _ in range(SINK_ITERS):
        m = m / (m.sum(axis=-1, keepdims=True) + np.float32(SINK_EPS))
        m = m / (m.sum(axis=-2, keepdims=True) + np.float32(SINK_EPS))
    return m * (np.float32(1.0) - np.eye(K, dtype=np.float32))


def _prep_inputs(desc, nv, W_fusion, b_fusion, W_slot, b_slot, G):
    """Build the per-core in_maps (host-side layout prep + sharding)."""
    desc = np.asarray(desc, np.float32)
    nv = np.asarray(nv, np.float32)
    W_fusion = np.asarray(W_fusion, np.float32)
    b_fusion = np.asarray(b_fusion, np.float32)
    W_slot = np.asarray(W_slot, np.float32)
    b_slot = np.asarray(b_slot, np.float32)

    descT = np.ascontiguousarray(desc.transpose(0, 2, 1)).reshape(B, 2, 128, N)
    nvT = np.ascontiguousarray(nv.transpose(0, 2, 1)).reshape(B, 2, 128, N)

    def wprep(w):  # [o, d] -> [p, c, o] with d = c*128 + p
        return np.ascontiguousarray(w.T.reshape(2, 128, D).transpose(1, 0, 2))

    w1t = wprep(W_fusion[:, :D])
    w2t = wprep(W_fusion[:, D:])
    wst = np.ascontiguousarray(W_slot.T.reshape(2, 128, K).transpose(1, 0, 2))
    bfu = np.ascontiguousarray(b_fusion.reshape(2, 128).T)
    bsl = np.ascontiguousarray(b_slot[:, None])

    gpad = np.zeros((K, 256), np.float32)
    for h in range(H):
        g, j = divmod(h, 4)
        gpad[:, 128 * g + 32 * j : 128 * g + 32 * j + K] = G[h] / np.float32(TAU_SLOT)
    rep = np.zeros((K, 128), np.float32)
    for j in range(4):
        rep[np.arange(K), 32 * j + np.arange(K)] = 1.0

    shared = {
        "w1t": w1t, "w2t": w2t, "wst": wst, "bfu": bfu, "bsl": bsl,
        "gpad": gpad, "rep": rep, "onesd": np.ones((K, K), np.float32),
    }
    in_maps = []
    for i in range(NCORES):
        sl = slice(i * BLOC, (i + 1) * BLOC)
        in_maps.append({"descT": descT[sl], "nvT": nvT[sl], **shared})
    return in_maps


def _host_regs(S, G):
    """Regularizer scalars from S [B,N,K] and G [H,K,K] (float32, as reference)."""
    eye = np.eye(K, dtype=np.float32)
    sts = np.einsum("bnk,bnl->bkl", S, S) / np.float32(N)
    offdiag = sts * (np.float32(1.0) - eye)
    reg_orth = np.float32(ORTH_LAMBDA) * np.mean(offdiag**2, dtype=np.float32)

    u = S.mean(axis=1)
    u = u / (u.sum(axis=-1, keepdims=True) + np.float32(EPS))
    uc = np.maximum(u, np.float32(EPS))
    kl = np.sum(uc * (np.log(uc) - np.log(np.float32(1.0 / K))), axis=-1)
    reg_usage = np.float32(USAGE_LAMBDA) * np.mean(kl, dtype=np.float32)

    v = G.reshape(H, -1)
    nrm = np.sqrt((v * v).sum(axis=1, keepdims=True))
    v = v / np.maximum(nrm, np.float32(1e-8))
    gram = v @ v.T
    g_reg = np.float32(FROB_LAMBDA) * (gram.sum() - np.trace(gram)) / (H * (H - 1))
    return np.float32(reg_orth + reg_usage + g_reg)


def run_on_device(in_maps, **kwargs):
    nc = _get_program()
    return run_bass_kernel_spmd(nc, in_maps, core_ids=list(range(NCORES)), **kwargs)


def kernel(
    desc_embeddings,
    name_value_embeddings,
    W_fusion,
    b_fusion,
    W_slot,
    b_slot,
    G_param,
):
    G = _host_G(G_param)
    in_maps = _prep_inputs(
        desc_embeddings, name_value_embeddings, W_fusion, b_fusion, W_slot, b_slot, G
    )
    res = run_on_device(in_maps)

    def unperm(name):
        a = np.concatenate([res.results[i][name] for i in range(NCORES)], axis=0)
        # [b, p, h*2+c, m] -> [b, h, c*128+p, m]
        a = a.reshape(B, 128, H, 2, N).transpose(0, 2, 3, 1, 4)
        return np.ascontiguousarray(a).reshape(B, H, N, N)

    Q = unperm("q_out")
    bias_log = unperm("bl_out")
    st = np.concatenate([res.results[i]["st_out"] for i in range(NCORES)], axis=0)
    S = np.ascontiguousarray(st.transpose(0, 2, 1))  # [B, N, K]

    total_reg = _host_regs(S, G)
    return bias_log, Q, total_reg
